# revision 6
# baseline (speedup 1.0000x reference)
"""Cosine-similarity (2-slot Hungarian-matched) loss on 8 Trainium2 cores.

Math (per sample b, slots i,j in {0,1}):
    cos[i,j] = <pred[b,i]/|pred[b,i]|, gt[b,j]/|gt[b,j]|>
    best = max(cos00+cos11, cos01+cos10)
    loss = mean_b(1 - best/2)

Distribution: pure data parallel -- B=32768 split into 8 shards of 4096.
Each core streams its 64 MiB shard through SBUF in 2 MiB tiles (6 tile-pairs
deep, ~415 GB/s); per 128-sample column ACT does 4 Square+accum norms and DVE
4 scalar_tensor_tensor dot-accums (one norm rotates to DVE every 6th column
to balance the two ~5.4us/column engine loads).  A chunked epilogue -- fused
stride-0-broadcast norm-product mul, rsqrt via exp(-0.5*ln), small elementwise
ops on the otherwise-idle GpSimd -- runs one chunk behind compute so nothing
stalls, and folds per-column sums into PSUM with TensorE ones-matmuls.  The
final scalar leaves through a single-descriptor 4-byte DMA (a [128,1] out DMA
costs ~8us in trickling per-partition HBM write receipts).  Tile teardown is
slimmed to drain-only; the Bass preamble re-clears semaphores each execution.
Host adds the 8 per-core scalars and finishes 1 - total/(2B).
"""

import sys

import numpy as np

sys.path.insert(0, "/opt/trn_rl_repo")

import bass_rust
import concourse.bacc as bacc
import concourse.mybir as mybir
import concourse.tile as tile
from concourse.bass_utils import run_bass_kernel_spmd
from concourse.vector_clock import ScopedClock

B, S, D = 32768, 2, 1024
N_CORES = 8
B_C = B // N_CORES
NPART = 128
NCOL = B_C // NPART          # 32 stat columns, one per 128 samples
F32 = mybir.dt.float32
AF = mybir.ActivationFunctionType
ALU = mybir.AluOpType

TILE_S = 256                 # samples per steady-state tile (2 MiB per tensor)
INPUT_BUFS = 6               # tile-pairs in flight (192 KiB/partition)
EPI_CHUNK = 8                # stat-columns per epilogue chunk
DVE_SHIFT = 5                # every Nth column, one norm runs on DVE
LEAD_PLAN = (128, 128)       # small first tiles for a fast lead-in


class SlimTileContext(tile.TileContext):
    """TileContext whose teardown is just the final drain: the semaphore-clear
    pass and both all-engine barriers are skipped.  Nothing runs after this
    context and the Bass preamble re-clears the kernel sem range on every
    execution, so resetting ~40 sems one EVENT_SEMAPHORE at a time (~5us of
    tail) is dead weight.  The drain still waits for every DMA to land."""

    def _drain_and_barrier(self, tick_clock, wait_clock):
        drain_inst = self.nc.sync.drain()
        wait_clock.add_sem_waits(
            drain_inst.ins, ScopedClock({None: tick_clock.global_clock})
        )
        popped = self.nc._tile_sem_poison_stack.pop()
        assert popped is self._sem_poison


def build_nc():
    plan = list(LEAD_PLAN)
    rem = B_C - sum(plan)
    plan += [TILE_S] * (rem // TILE_S)
    max_row = max(plan) // NPART * S * D

    nc = bacc.Bacc(trn_type="TRN2")
    pred_h = nc.declare_dram_parameter("pred", [B_C, S, D], F32, isOutput=False)
    gt_h = nc.declare_dram_parameter("gt", [B_C, S, D], F32, isOutput=False)
    out_h = nc.declare_dram_parameter("out", [1, 1], F32, isOutput=True)

    with SlimTileContext(nc) as tc:
        with (
            tc.tile_pool(name="pin", bufs=INPUT_BUFS) as pin,
            tc.tile_pool(name="stats", bufs=1) as stats,
            tc.tile_pool(name="scratch", bufs=1) as scratch,
            tc.tile_pool(name="epi", bufs=1) as epi,
            tc.tile_pool(name="ps", bufs=1, space="PSUM") as ps,
        ):
            # norms: [np0 | np1 | ng0 | ng1]; crosses: [c00 | c01 | c10 | c11]
            st_n = stats.tile([NPART, 4, NCOL], F32, tag="st_n", name="st_n")
            st_c = stats.tile([NPART, 4, NCOL], F32, tag="st_c", name="st_c")

            scr_a = scratch.tile([NPART, D], F32, tag="scr_a", name="scr_a")
            scr_v = scratch.tile([NPART, D], F32, tag="scr_v", name="scr_v")
            ones = scratch.tile([NPART, 1], F32, tag="ones", name="ones")
            nc.vector.memset(ones[:], 1.0)

            t_all = epi.tile([NPART, 4, EPI_CHUNK], F32, tag="t_all", name="t_all")
            cos_all = epi.tile([NPART, 4, EPI_CHUNK], F32, tag="cos", name="cos")
            s_id = epi.tile([NPART, EPI_CHUNK], F32, tag="s_id", name="s_id")
            s_sw = epi.tile([NPART, EPI_CHUNK], F32, tag="s_sw", name="s_sw")
            best = epi.tile([NPART, EPI_CHUNK], F32, tag="best", name="best")
            res = epi.tile([1, 1], F32, tag="res", name="res")
            acc = ps.tile([1, EPI_CHUNK], F32, tag="acc", name="acc")

            # chunk plan: EPI_CHUNK-sized chunks, the last split 4+2+2 so
            # almost no epilogue work remains after the final column lands
            bounds = []
            c = 0
            while c + EPI_CHUNK < NCOL:
                bounds.append((c, c + EPI_CHUNK))
                c += EPI_CHUNK
            for w in (EPI_CHUNK // 2, EPI_CHUNK // 4, EPI_CHUNK // 4):
                bounds.append((c, c + w))
                c += w
            assert c == NCOL
            n_chunks = len(bounds)

            def bcast(view, pos):
                """Insert a stride-0 broadcast dim of size 2 at `pos`."""
                ap = list(view.ap)
                ap.insert(pos, [0, 2])
                return bass_rust.AP(view.tensor, view.offset, ap)

            def epilogue(ci):
                c0, c1 = bounds[ci]
                w = c1 - c0
                # t_all[p,i,j,:] = np_i * ng_j in ONE op via stride-0 APs;
                # small elementwise work rides on the otherwise-idle GpSimd
                nc.gpsimd.tensor_tensor(
                    out=t_all[:, :, :w].rearrange("p (i j) w -> p i j w", i=2),
                    in0=bcast(st_n[:, 0:2, c0:c1], 2),
                    in1=bcast(st_n[:, 2:4, c0:c1], 1),
                    op=ALU.mult,
                )
                # rsqrt via exp(-0.5*ln(t))
                nc.scalar.activation(t_all[:, :, :w], t_all[:, :, :w], AF.Ln)
                nc.scalar.activation(t_all[:, :, :w], t_all[:, :, :w], AF.Exp,
                                     scale=-0.5)
                nc.gpsimd.tensor_mul(cos_all[:, :, :w], st_c[:, :, c0:c1],
                                     t_all[:, :, :w])
                nc.gpsimd.tensor_add(s_id[:, :w], cos_all[:, 0, :w],
                                     cos_all[:, 3, :w])
                nc.gpsimd.tensor_add(s_sw[:, :w], cos_all[:, 1, :w],
                                     cos_all[:, 2, :w])
                # Pool TT supports add/mult only -> max stays on DVE
                nc.vector.tensor_max(best[:, :w], s_id[:, :w], s_sw[:, :w])
                # cross-partition reduce on the idle TensorE, accumulating
                # chunk sums into one PSUM row
                nc.tensor.matmul(
                    acc[:, :w], ones[:], best[:, :w],
                    start=(ci == 0), stop=(ci == n_chunks - 1),
                )

            def compute_col(p_t, g_t, j, col):
                p0 = p_t[:, (j * S + 0) * D:(j * S + 1) * D]
                p1 = p_t[:, (j * S + 1) * D:(j * S + 2) * D]
                g0 = g_t[:, (j * S + 0) * D:(j * S + 1) * D]
                g1 = g_t[:, (j * S + 1) * D:(j * S + 2) * D]
                norm_srcs = (p0, p1, g0, g1)
                # every DVE_SHIFTth column one g-norm runs on DVE: balances
                # ACT (4x1365ns) against DVE (4x1267ns + epilogue max)
                shift = col % DVE_SHIFT == DVE_SHIFT // 2
                for k in range(4):
                    if k == 3 and shift:
                        nc.vector.scalar_tensor_tensor(
                            out=scr_v[:], in0=norm_srcs[k], scalar=1.0,
                            in1=norm_srcs[k], op0=ALU.mult, op1=ALU.mult,
                            accum_out=st_n[:, k, col:col + 1],
                        )
                    else:
                        nc.scalar.activation(
                            scr_a[:], norm_srcs[k], AF.Square,
                            accum_out=st_n[:, k, col:col + 1],
                        )
                for idx, (pi, gj) in enumerate(((p0, g0), (p0, g1), (p1, g0),
                                                (p1, g1))):
                    nc.vector.scalar_tensor_tensor(
                        out=scr_v[:], in0=pi, scalar=1.0, in1=gj,
                        op0=ALU.mult, op1=ALU.mult,
                        accum_out=st_c[:, idx, col:col + 1],
                    )

            samp0 = 0
            next_chunk = 0
            for i, ts in enumerate(plan):
                nsub = ts // NPART
                row = nsub * S * D
                p_t = pin.tile([NPART, max_row], F32, tag="P", name="P")
                g_t = pin.tile([NPART, max_row], F32, tag="G", name="G")
                p_src = pred_h[samp0:samp0 + ts].rearrange(
                    "(p n) s d -> p (n s d)", p=NPART, n=nsub)
                g_src = gt_h[samp0:samp0 + ts].rearrange(
                    "(p n) s d -> p (n s d)", p=NPART, n=nsub)
                # during the ramp, g descriptors generate on the second HWDGE
                # ring (ACT queue) in parallel with p on the Sync ring; ACT is
                # idle until the first tile lands anyway
                g_dma = nc.scalar.dma_start if i < 3 else nc.sync.dma_start
                if i < 3:
                    # ramp: half-tile DMAs on both HWDGE rings engage ~4 DMA
                    # queues at once -- a single early DMA only streams at
                    # ~340 GB/s vs ~415 once several queues are in flight
                    h = row // 2
                    for o in (0, h):
                        nc.sync.dma_start(out=p_t[:, o:o + h],
                                          in_=p_src[:, o:o + h])
                        g_dma(out=g_t[:, o:o + h], in_=g_src[:, o:o + h])
                elif i == len(plan) - 1:
                    # per-column DMAs: the tail compute starts per column
                    for j in range(nsub):
                        o = j * S * D
                        nc.sync.dma_start(out=p_t[:, o:o + S * D],
                                          in_=p_src[:, o:o + S * D])
                        g_dma(out=g_t[:, o:o + S * D],
                              in_=g_src[:, o:o + S * D])
                else:
                    nc.sync.dma_start(out=p_t[:, :row], in_=p_src)
                    g_dma(out=g_t[:, :row], in_=g_src)

                if i == 0:
                    # ACT table set 6 (square+ln+exp) loads AFTER tile 0's
                    # g-DMA triggers on the ACT HWDGE ring: the ~2.7us
                    # TABLE_LOAD+drain otherwise delays the first tile's
                    # descriptor generation; the table is still resident
                    # ~1us before the first SQUARE can have data
                    nc.scalar.add_instruction(
                        mybir.InstLoadActFuncSet(
                            name=nc.get_next_instruction_name(),
                            act_func_set_id=6,
                            ins=[],
                            outs=[],
                        )
                    )
                for j in range(nsub):
                    col = samp0 // NPART + j
                    # each chunk's epilogue runs one chunk late: its stats are
                    # long-finished, so neither DVE nor ACT stalls on it
                    if next_chunk < n_chunks and col >= bounds[next_chunk][1] + 2:
                        epilogue(next_chunk)
                        next_chunk += 1
                    compute_col(p_t, g_t, j, col)
                samp0 += ts
            while next_chunk < n_chunks:
                epilogue(next_chunk)
                next_chunk += 1

            nc.vector.reduce_sum(res[:], acc[:], axis=mybir.AxisListType.X)
            nc.sync.dma_start(out=out_h[:], in_=res[:])
    nc.finalize()
    return nc


_CACHE = {}


def _get_nc():
    if "nc" not in _CACHE:
        _CACHE["nc"] = build_nc()
    return _CACHE["nc"]


def run_spmd(pred, gt, nc=None, **kwargs):
    """Run the SPMD kernel; returns BassKernelResults (per-core scalars)."""
    pred = np.ascontiguousarray(np.asarray(pred), dtype=np.float32)
    gt = np.ascontiguousarray(np.asarray(gt), dtype=np.float32)
    assert pred.shape == (B, S, D) and gt.shape == (B, S, D)
    if nc is None:
        nc = _get_nc()
    in_maps = [
        {"pred": pred[c * B_C:(c + 1) * B_C], "gt": gt[c * B_C:(c + 1) * B_C]}
        for c in range(N_CORES)
    ]
    return run_bass_kernel_spmd(nc, in_maps, list(range(N_CORES)), **kwargs)


def _total(res):
    return sum(float(np.sum(r["out"], dtype=np.float64)) for r in res.results)


def kernel(pred, gt):
    res = run_spmd(pred, gt)
    loss = 1.0 - _total(res) / (2.0 * B)
    return np.array(loss, dtype=np.float32)


# revision 7
# speedup vs baseline: 1.1733x; 1.1733x over previous
"""Cosine-similarity (2-slot Hungarian-matched) loss on 8 Trainium2 cores.

Math (per sample b, slots i,j in {0,1}):
    cos[i,j] = <pred[b,i]/|pred[b,i]|, gt[b,j]/|gt[b,j]|>
    best = max(cos00+cos11, cos01+cos10)
    loss = mean_b(1 - best/2)

Distribution: pure data parallel -- B=32768 split into 8 shards of 4096.
Each core streams its 64 MiB shard through SBUF in 2 MiB tiles (6 tile-pairs
deep, ~415 GB/s); per 128-sample column ACT does 4 Square+accum norms and DVE
4 scalar_tensor_tensor dot-accums (one norm rotates to DVE every 6th column
to balance the two ~5.4us/column engine loads).  A chunked epilogue -- fused
stride-0-broadcast norm-product mul, rsqrt via exp(-0.5*ln), small elementwise
ops on the otherwise-idle GpSimd -- runs one chunk behind compute so nothing
stalls, and folds per-column sums into PSUM with TensorE ones-matmuls.  The
final scalar leaves through a single-descriptor 4-byte DMA (a [128,1] out DMA
costs ~8us in trickling per-partition HBM write receipts).  Tile teardown is
slimmed to drain-only; the Bass preamble re-clears semaphores each execution.
Host adds the 8 per-core scalars and finishes 1 - total/(2B).
"""

import sys

import numpy as np

sys.path.insert(0, "/opt/trn_rl_repo")

import bass_rust
import concourse.bacc as bacc
import concourse.mybir as mybir
import concourse.tile as tile
from concourse.bass_utils import run_bass_kernel_spmd
from concourse.vector_clock import ScopedClock

B, S, D = 32768, 2, 1024
N_CORES = 8
B_C = B // N_CORES
NPART = 128
NCOL = B_C // NPART          # 32 stat columns, one per 128 samples
F32 = mybir.dt.float32
AF = mybir.ActivationFunctionType
ALU = mybir.AluOpType

TILE_S = 256                 # samples per steady-state tile (2 MiB per tensor)
INPUT_BUFS = 6               # tile-pairs in flight (192 KiB/partition)
EPI_CHUNK = 8                # stat-columns per epilogue chunk
DVE_SHIFT = 5                # every Nth column, one norm runs on DVE
LEAD_PLAN = (128, 128)       # small first tiles for a fast lead-in


class SlimTileContext(tile.TileContext):
    """TileContext whose teardown is just the final drain: the semaphore-clear
    pass and both all-engine barriers are skipped.  Nothing runs after this
    context and the Bass preamble re-clears the kernel sem range on every
    execution, so resetting ~40 sems one EVENT_SEMAPHORE at a time (~5us of
    tail) is dead weight.  The drain still waits for every DMA to land."""

    def _drain_and_barrier(self, tick_clock, wait_clock):
        drain_inst = self.nc.sync.drain()
        wait_clock.add_sem_waits(
            drain_inst.ins, ScopedClock({None: tick_clock.global_clock})
        )
        popped = self.nc._tile_sem_poison_stack.pop()
        assert popped is self._sem_poison


def build_nc():
    plan = list(LEAD_PLAN)
    rem = B_C - sum(plan)
    plan += [TILE_S] * (rem // TILE_S)
    max_row = max(plan) // NPART * S * D

    nc = bacc.Bacc(trn_type="TRN2")
    pred_h = nc.declare_dram_parameter("pred", [B_C, S, D], F32, isOutput=False)
    gt_h = nc.declare_dram_parameter("gt", [B_C, S, D], F32, isOutput=False)
    out_h = nc.declare_dram_parameter("out", [1, 1], F32, isOutput=True)

    with SlimTileContext(nc) as tc:
        with (
            tc.tile_pool(name="pin", bufs=INPUT_BUFS) as pin,
            tc.tile_pool(name="stats", bufs=1) as stats,
            tc.tile_pool(name="scratch", bufs=1) as scratch,
            tc.tile_pool(name="epi", bufs=1) as epi,
            tc.tile_pool(name="ps", bufs=1, space="PSUM") as ps,
        ):
            # norms: [np0 | np1 | ng0 | ng1]; crosses: [c00 | c01 | c10 | c11]
            st_n = stats.tile([NPART, 4, NCOL], F32, tag="st_n", name="st_n")
            st_c = stats.tile([NPART, 4, NCOL], F32, tag="st_c", name="st_c")

            scr_a = scratch.tile([NPART, D], F32, tag="scr_a", name="scr_a")
            scr_v = scratch.tile([NPART, D], F32, tag="scr_v", name="scr_v")
            ones = scratch.tile([NPART, 1], F32, tag="ones", name="ones")
            nc.vector.memset(ones[:], 1.0)

            t_all = epi.tile([NPART, 4, EPI_CHUNK], F32, tag="t_all", name="t_all")
            cos_all = epi.tile([NPART, 4, EPI_CHUNK], F32, tag="cos", name="cos")
            s_id = epi.tile([NPART, EPI_CHUNK], F32, tag="s_id", name="s_id")
            s_sw = epi.tile([NPART, EPI_CHUNK], F32, tag="s_sw", name="s_sw")
            best = epi.tile([NPART, EPI_CHUNK], F32, tag="best", name="best")
            res = epi.tile([1, 1], F32, tag="res", name="res")
            acc = ps.tile([1, EPI_CHUNK], F32, tag="acc", name="acc")

            # chunk plan: EPI_CHUNK-sized chunks, the last split 4+2+2 so
            # almost no epilogue work remains after the final column lands
            bounds = []
            c = 0
            while c + EPI_CHUNK < NCOL:
                bounds.append((c, c + EPI_CHUNK))
                c += EPI_CHUNK
            for w in (EPI_CHUNK // 2, EPI_CHUNK // 4, EPI_CHUNK // 4):
                bounds.append((c, c + w))
                c += w
            assert c == NCOL
            n_chunks = len(bounds)

            def bcast(view, pos):
                """Insert a stride-0 broadcast dim of size 2 at `pos`."""
                ap = list(view.ap)
                ap.insert(pos, [0, 2])
                return bass_rust.AP(view.tensor, view.offset, ap)

            def epilogue(ci):
                c0, c1 = bounds[ci]
                w = c1 - c0
                # t_all[p,i,j,:] = np_i * ng_j in ONE op via stride-0 APs;
                # small elementwise work rides on the otherwise-idle GpSimd
                nc.gpsimd.tensor_tensor(
                    out=t_all[:, :, :w].rearrange("p (i j) w -> p i j w", i=2),
                    in0=bcast(st_n[:, 0:2, c0:c1], 2),
                    in1=bcast(st_n[:, 2:4, c0:c1], 1),
                    op=ALU.mult,
                )
                # rsqrt via exp(-0.5*ln(t))
                nc.scalar.activation(t_all[:, :, :w], t_all[:, :, :w], AF.Ln)
                nc.scalar.activation(t_all[:, :, :w], t_all[:, :, :w], AF.Exp,
                                     scale=-0.5)
                nc.gpsimd.tensor_mul(cos_all[:, :, :w], st_c[:, :, c0:c1],
                                     t_all[:, :, :w])
                nc.gpsimd.tensor_add(s_id[:, :w], cos_all[:, 0, :w],
                                     cos_all[:, 3, :w])
                nc.gpsimd.tensor_add(s_sw[:, :w], cos_all[:, 1, :w],
                                     cos_all[:, 2, :w])
                # Pool TT supports add/mult only -> max stays on DVE
                nc.vector.tensor_max(best[:, :w], s_id[:, :w], s_sw[:, :w])
                # cross-partition reduce on the idle TensorE, accumulating
                # chunk sums into one PSUM row
                nc.tensor.matmul(
                    acc[:, :w], ones[:], best[:, :w],
                    start=(ci == 0), stop=(ci == n_chunks - 1),
                )

            def compute_col(p_t, g_t, j, col):
                p0 = p_t[:, (j * S + 0) * D:(j * S + 1) * D]
                p1 = p_t[:, (j * S + 1) * D:(j * S + 2) * D]
                g0 = g_t[:, (j * S + 0) * D:(j * S + 1) * D]
                g1 = g_t[:, (j * S + 1) * D:(j * S + 2) * D]
                norm_srcs = (p0, p1, g0, g1)
                # every DVE_SHIFTth column one g-norm runs on DVE: balances
                # ACT (4x1365ns) against DVE (4x1267ns + epilogue max).
                # No shifts during the ramp (DVE is data-starved there) or
                # in the last columns (DVE paces the tail).
                shift = 6 <= col < 30 and col % DVE_SHIFT == 1
                for k in range(4):
                    if k == 3 and shift:
                        nc.vector.scalar_tensor_tensor(
                            out=scr_v[:], in0=norm_srcs[k], scalar=1.0,
                            in1=norm_srcs[k], op0=ALU.mult, op1=ALU.mult,
                            accum_out=st_n[:, k, col:col + 1],
                        )
                    else:
                        nc.scalar.activation(
                            scr_a[:], norm_srcs[k], AF.Square,
                            accum_out=st_n[:, k, col:col + 1],
                        )
                for idx, (pi, gj) in enumerate(((p0, g0), (p0, g1), (p1, g0),
                                                (p1, g1))):
                    nc.vector.scalar_tensor_tensor(
                        out=scr_v[:], in0=pi, scalar=1.0, in1=gj,
                        op0=ALU.mult, op1=ALU.mult,
                        accum_out=st_c[:, idx, col:col + 1],
                    )

            samp0 = 0
            next_chunk = 0
            for i, ts in enumerate(plan):
                nsub = ts // NPART
                row = nsub * S * D
                p_t = pin.tile([NPART, max_row], F32, tag="P", name="P")
                g_t = pin.tile([NPART, max_row], F32, tag="G", name="G")
                p_src = pred_h[samp0:samp0 + ts].rearrange(
                    "(p n) s d -> p (n s d)", p=NPART, n=nsub)
                g_src = gt_h[samp0:samp0 + ts].rearrange(
                    "(p n) s d -> p (n s d)", p=NPART, n=nsub)
                # during the ramp, g descriptors generate on the second HWDGE
                # ring (ACT queue) in parallel with p on the Sync ring; ACT is
                # idle until the first tile lands anyway
                g_dma = nc.scalar.dma_start if i < 3 else nc.sync.dma_start
                if i < 3:
                    # ramp: half-tile DMAs on both HWDGE rings engage ~4 DMA
                    # queues at once -- a single early DMA only streams at
                    # ~340 GB/s vs ~415 once several queues are in flight
                    h = row // 2
                    for o in (0, h):
                        nc.sync.dma_start(out=p_t[:, o:o + h],
                                          in_=p_src[:, o:o + h])
                        g_dma(out=g_t[:, o:o + h], in_=g_src[:, o:o + h])
                elif i == len(plan) - 1:
                    # per-column DMAs: the tail compute starts per column
                    for j in range(nsub):
                        o = j * S * D
                        nc.sync.dma_start(out=p_t[:, o:o + S * D],
                                          in_=p_src[:, o:o + S * D])
                        g_dma(out=g_t[:, o:o + S * D],
                              in_=g_src[:, o:o + S * D])
                else:
                    nc.sync.dma_start(out=p_t[:, :row], in_=p_src)
                    g_dma(out=g_t[:, :row], in_=g_src)

                if i == 0:
                    # ACT table set 6 (square+ln+exp) loads AFTER tile 0's
                    # g-DMA triggers on the ACT HWDGE ring: the ~2.7us
                    # TABLE_LOAD+drain otherwise delays the first tile's
                    # descriptor generation; the table is still resident
                    # ~1us before the first SQUARE can have data
                    nc.scalar.add_instruction(
                        mybir.InstLoadActFuncSet(
                            name=nc.get_next_instruction_name(),
                            act_func_set_id=6,
                            ins=[],
                            outs=[],
                        )
                    )
                for j in range(nsub):
                    col = samp0 // NPART + j
                    # each chunk's epilogue runs one chunk late: its stats are
                    # long-finished, so neither DVE nor ACT stalls on it
                    if next_chunk < n_chunks and col >= bounds[next_chunk][1] + 2:
                        epilogue(next_chunk)
                        next_chunk += 1
                    compute_col(p_t, g_t, j, col)
                samp0 += ts
            while next_chunk < n_chunks:
                epilogue(next_chunk)
                next_chunk += 1

            nc.vector.reduce_sum(res[:], acc[:], axis=mybir.AxisListType.X)
            nc.sync.dma_start(out=out_h[:], in_=res[:])
    nc.finalize()
    return nc


_CACHE = {}


def _get_nc():
    if "nc" not in _CACHE:
        _CACHE["nc"] = build_nc()
    return _CACHE["nc"]


def run_spmd(pred, gt, nc=None, **kwargs):
    """Run the SPMD kernel; returns BassKernelResults (per-core scalars)."""
    pred = np.ascontiguousarray(np.asarray(pred), dtype=np.float32)
    gt = np.ascontiguousarray(np.asarray(gt), dtype=np.float32)
    assert pred.shape == (B, S, D) and gt.shape == (B, S, D)
    if nc is None:
        nc = _get_nc()
    in_maps = [
        {"pred": pred[c * B_C:(c + 1) * B_C], "gt": gt[c * B_C:(c + 1) * B_C]}
        for c in range(N_CORES)
    ]
    return run_bass_kernel_spmd(nc, in_maps, list(range(N_CORES)), **kwargs)


def _total(res):
    return sum(float(np.sum(r["out"], dtype=np.float64)) for r in res.results)


def kernel(pred, gt):
    res = run_spmd(pred, gt)
    loss = 1.0 - _total(res) / (2.0 * B)
    return np.array(loss, dtype=np.float32)
